# revision 2
# baseline (speedup 1.0000x reference)
"""Trainium2 Bass kernel v2 for nn_Discriminator (minibatch discrimination).

Block-symmetric ("triangle") scheme over 8 cores. Global rows split into 8
blocks of 512. Core c computes D[i, j] for its own rows i (block c) against
j in blocks c..c+4 (local j-window [0, 2560), thanks to a per-core roll of
x). Each unordered block pair is computed once (d=4 twice, counted once):

  out_T[i in c]   <- own partial: sum_{j in window} exp(-D[i,j])  (ACT accum)
  out_T[j in c+d] <- transposed partial, d=1..3: sum_{i in c} exp(-D[i,j]),
                     accumulated on-chip via identity-matmul into PSUM.

Host combines own + 3 transposed partials per block and applies the tiny
sigmoid head (z = h @ W3h + out_T @ W3o + b3) in numpy (fp64).

Per-core pipeline (512 rows, j-window JW=2560, processed in 2 column
phases 1536+1024 to fit SBUF):
  - MLP in fp32 (keeps final rel err ~2e-3): h = relu-MLP(x);
    Ms_T = T_perm^T @ h stored fp16 per phase [128, 16, 1536] plus fp32
    own-row scalars [128, 16, 512].
  - 16 k-tiles of |Ms[:,j] - Ms[:,i]| split across engines per row:
      k0-7  : DVE custom pair op (|A-a|+|B-b|) -> mm tiles m0-3
      k8,9  : ACT Abs (pair layout), folded by DVE tensor_tensor add -> m4
      k10-12: ACT Abs plain -> m5-7
      k13-15: Pool tensor_scalar(sub, abs_max) plain -> m8-10
  - Mask matmuls reduce channels: 11 mm tiles accumulate D in PSUM,
    2 rows packed per PSUM bank set (partitions 0-63 / 64-127).
  - ACT exp(-D) per 512-chunk with accum_out -> own partials; chunks 1-3
    feed identity matmuls accumulating transposed partials in PSUM.
"""

import numpy as np

import concourse.bass as bass
import concourse.bacc as bacc
import concourse.tile as tile
from concourse import mybir
import concourse.dve_ops as dve_ops_mod
from concourse.dve_ops import DveOp
from concourse.dve_spec import Spec, Src0, Src1, C0, C1, lower, maxx
from concourse.dve_uop import DveOpSpec
from concourse.bass_utils import run_bass_kernel_spmd

F32 = mybir.dt.float32
FP16 = mybir.dt.float16

N = 4096
D_IN = 100
H1 = 200
H2 = 100
NB = 64
NCH = 32
KT = 16
NMM = 10
N_CORES = 8
ROWS = N // N_CORES      # 512 own rows per core
NBLK = 5                 # j blocks per core (d = 0..4)
JW = NBLK * ROWS         # 2560 j-window
CHUNK = 512              # PSUM bank chunk
NCHUNK = JW // CHUNK     # 5
TPW = 3 * ROWS           # transposed-partial width (d=1..3)
PH0 = 3 * CHUNK          # phase A columns [0, 1536)
PHASES = ((0, PH0, (0, 1, 2)), (PH0, JW, (3, 4)))


def register_absdiff_pair():
    """Custom DVE op |in0-s0| + |in1-s1| (registered once per process)."""
    name = "ABS_DIFF_PAIR_ANT"
    if name in dve_ops_mod._SUB_OPCODE_FOR_NAME:
        return next(o for o in dve_ops_mod.OPS if o.name == name)
    body = maxx(Src0 - C0, C0 - Src0) + maxx(Src1 - C1, C1 - Src1)

    def ref(in0, in1, s0, s1, imm2):
        return np.abs(in0.astype(np.float32) - s0) + np.abs(in1.astype(np.float32) - s1)

    spec = Spec(body=body, reference=ref)
    opcode = dve_ops_mod._CUSTOM_DVE_ROW_BASE + len(dve_ops_mod.OPS)
    uops = lower(spec, ver="v3")
    sha = DveOpSpec(name=name, opcode=opcode, uops=uops, rd1_en=True).sha("v3")
    op = DveOp(name, spec, subdim=False, uops_sha={"v3": sha})
    dve_ops_mod.OPS.append(op)
    dve_ops_mod.CUSTOM_DVE_SPECS[name] = spec
    dve_ops_mod._SUB_OPCODE_FOR_NAME[name] = opcode
    return op


ABS_DIFF_PAIR = register_absdiff_pair()


def kt_bc(k, p):
    """(b, c) carried by partition p of k-tile k."""
    if k < 12:
        return 8 * (k // 2) + p // 16, (p % 16) + 16 * (k % 2)
    return 48 + 4 * (k - 12) + p // 32, p % 32


def t_col_perm():
    cols = []
    for k in range(KT):
        for p in range(128):
            b, c = kt_bc(k, p)
            cols.append(b * NCH + c)
    cols = np.array(cols)
    assert sorted(cols.tolist()) == list(range(NB * NCH))
    return cols


def mm_tile_b(m, p):
    """Output b-kernel for partition p of mm tile m."""
    if m < 6:
        return 8 * m + p // 16
    return 48 + 4 * (m - 6) + p // 32


def make_masks():
    m = np.zeros((128, NMM, NB), dtype=np.float16)
    for mm in range(NMM):
        for p in range(128):
            m[p, mm, mm_tile_b(mm, p)] = 1.0
    return m


def build_nc(n_i=ROWS, repeat=1):
    """Per-core Bass program (SPMD across the 8 cores). n_i = own rows."""
    assert n_i % 2 == 0
    packs = n_i // 2
    nc = bacc.Bacc("TRN2", target_bir_lowering=False, debug=False)

    xT = nc.dram_tensor("xT", [D_IN, JW], F32, kind="ExternalInput")
    w1a = nc.dram_tensor("w1a", [D_IN, 128], F32, kind="ExternalInput")
    w1b = nc.dram_tensor("w1b", [D_IN, H1 - 128], F32, kind="ExternalInput")
    b1a = nc.dram_tensor("b1a", [128, 1], F32, kind="ExternalInput")
    b1b = nc.dram_tensor("b1b", [H1 - 128, 1], F32, kind="ExternalInput")
    w2a = nc.dram_tensor("w2a", [128, H2], F32, kind="ExternalInput")
    w2b = nc.dram_tensor("w2b", [H1 - 128, H2], F32, kind="ExternalInput")
    b2 = nc.dram_tensor("b2", [H2, 1], F32, kind="ExternalInput")
    tperm = nc.dram_tensor("tperm", [D_IN, NB * NCH], F32, kind="ExternalInput")
    masks = nc.dram_tensor("masks", [128, NMM, NB], FP16, kind="ExternalInput")
    ident = nc.dram_tensor("ident", [128, 128], FP16, kind="ExternalInput")

    rs_out = nc.dram_tensor("rs_out", [128, packs], F32, kind="ExternalOutput")
    tpo = nc.dram_tensor("tpo", [128, TPW], F32, kind="ExternalOutput")
    ho = nc.dram_tensor("ho", [D_IN, ROWS], F32, kind="ExternalOutput")

    with tile.TileContext(nc) as tc:
        with (
            tc.tile_pool(name="const", bufs=1) as const,
        ):
            mst = const.tile([128, KT, PH0], FP16, tag="mst")
            msc32 = const.tile([128, KT, ROWS], F32, tag="msc32")
            h_all = const.tile([D_IN, JW], F32, tag="hall")
            masks_sb = const.tile([128, NMM, NB], FP16, tag="masks")
            ident_sb = const.tile([128, 128], FP16, tag="ident")
            rs = const.tile([128, packs, NCHUNK], F32, tag="rs")

            nc.sync.dma_start(masks_sb[:], masks[:])
            nc.sync.dma_start(ident_sb[:], ident[:])

            # ---------------- MLP h (fp32) ----------------
            with (
                tc.tile_pool(name="mlp_w", bufs=1) as mlp_w,
                tc.tile_pool(name="mlp_x", bufs=2) as mlp_x,
                tc.tile_pool(name="mlp_h", bufs=2) as mlp_h,
                tc.tile_pool(name="mlp_ps", bufs=4, space="PSUM") as mlp_ps,
            ):
                w1a_sb = mlp_w.tile([D_IN, 128], F32, tag="w1a")
                w1b_sb = mlp_w.tile([D_IN, H1 - 128], F32, tag="w1b")
                w2a_sb = mlp_w.tile([128, H2], F32, tag="w2a")
                w2b_sb = mlp_w.tile([H1 - 128, H2], F32, tag="w2b")
                b1a_sb = mlp_w.tile([128, 1], F32, tag="b1a")
                b1b_sb = mlp_w.tile([H1 - 128, 1], F32, tag="b1b")
                b2_sb = mlp_w.tile([H2, 1], F32, tag="b2")
                nc.sync.dma_start(w1a_sb[:], w1a[:])
                nc.sync.dma_start(w1b_sb[:], w1b[:])
                nc.sync.dma_start(w2a_sb[:], w2a[:])
                nc.sync.dma_start(w2b_sb[:], w2b[:])
                nc.sync.dma_start(b1a_sb[:], b1a[:])
                nc.sync.dma_start(b1b_sb[:], b1b[:])
                nc.sync.dma_start(b2_sb[:], b2[:])

                for jc in range(NCHUNK):
                    js = slice(jc * CHUNK, (jc + 1) * CHUNK)
                    x_c = mlp_x.tile([D_IN, CHUNK], F32, tag="xc")
                    nc.sync.dma_start(x_c[:], xT[:, js])
                    ps1 = mlp_ps.tile([128, CHUNK], F32, tag="ps")
                    nc.tensor.matmul(ps1[:], lhsT=w1a_sb[:], rhs=x_c[:],
                                     start=True, stop=True)
                    h1a = mlp_h.tile([128, CHUNK], F32, tag="h1a")
                    nc.scalar.activation(h1a[:], ps1[:],
                                         mybir.ActivationFunctionType.Relu,
                                         bias=b1a_sb[:], scale=1.0)
                    ps2 = mlp_ps.tile([128, CHUNK], F32, tag="ps")
                    nc.tensor.matmul(ps2[0:H1 - 128, :], lhsT=w1b_sb[:],
                                     rhs=x_c[:], start=True, stop=True)
                    h1b = mlp_h.tile([H1 - 128, CHUNK], F32, tag="h1b")
                    nc.scalar.activation(h1b[:], ps2[0:H1 - 128, :],
                                         mybir.ActivationFunctionType.Relu,
                                         bias=b1b_sb[:], scale=1.0)
                    ps3 = mlp_ps.tile([128, CHUNK], F32, tag="ps")
                    nc.tensor.matmul(ps3[0:H2, :], lhsT=w2a_sb[:], rhs=h1a[:],
                                     start=True, stop=False)
                    nc.tensor.matmul(ps3[0:H2, :], lhsT=w2b_sb[:], rhs=h1b[:],
                                     start=False, stop=True)
                    nc.scalar.activation(h_all[:, js], ps3[0:H2, :],
                                         mybir.ActivationFunctionType.Relu,
                                         bias=b2_sb[:], scale=1.0)
                nc.sync.dma_start(ho[:], h_all[:, 0:ROWS])

            # ---------------- pairwise phase ----------------
            with (
                tc.tile_pool(name="l3w", bufs=1) as l3w,
                tc.tile_pool(name="abs_t", bufs=26) as abs_t,    # f16 thirds
                tc.tile_pool(name="abs_f", bufs=12) as abs_f,    # ACT tiles
                tc.tile_pool(name="epool", bufs=6) as epool,
                tc.tile_pool(name="l3ps", bufs=2, space="PSUM") as l3ps,
                tc.tile_pool(name="dps", bufs=3, space="PSUM") as dps_pool,
                tc.tile_pool(name="tpps", bufs=1, space="PSUM") as tp_pool,
            ):
                t_sb = l3w.tile([D_IN, NB * NCH], F32, tag="tsb")
                nc.sync.dma_start(t_sb[:], tperm[:])
                tp_ps = tp_pool.tile([128, TPW], F32, tag="tp")

                for _rep in range(repeat):
                  for ph, (p0, p1, chunks) in enumerate(PHASES):
                    pw = p1 - p0
                    # L3 for this phase's columns: mst[:, k, 0:pw]
                    for k in range(KT):
                        for jc in chunks:
                            js = slice(jc * CHUNK, (jc + 1) * CHUNK)
                            ls = slice(jc * CHUNK - p0, (jc + 1) * CHUNK - p0)
                            ps = l3ps.tile([128, CHUNK], F32, tag="l3p")
                            nc.tensor.matmul(ps[:],
                                             lhsT=t_sb[:, k * 128:(k + 1) * 128],
                                             rhs=h_all[:, js],
                                             start=True, stop=True)
                            if (k + jc) % 2 == 0:
                                nc.vector.tensor_copy(mst[:, k, ls], ps[:])
                            else:
                                nc.scalar.copy(mst[:, k, ls], ps[:])
                            if ph == 0 and jc == 0:
                                nc.vector.tensor_copy(msc32[:, k, :], ps[:])

                    # per-phase thirds (aligned to 512-chunks)
                    tparts = [(0, slice(0, 1024), 1024), (1, slice(1024, pw), pw - 1024)] \
                        if pw > 1024 else [(0, slice(0, pw), pw)]

                    for q in range(packs):
                        # mm_ops[r01][local_chunk] -> list of (mask_m, rhs AP)
                        mm_ops = []
                        for r01 in range(2):
                            i = 2 * q + r01
                            per_chunk = [[] for _ in chunks]
                            # m0-4: DVE pair ops (k0-9), all phase columns
                            for t in range(5):
                                th = [abs_t.tile([128, 1024], FP16, tag="at",
                                                 name="at") for _ in tparts]
                                for ti, js, w in tparts:
                                    nc.vector._custom_dve(
                                        ABS_DIFF_PAIR, out=th[ti][:, 0:w],
                                        in0=mst[:, 2 * t, js],
                                        in1=mst[:, 2 * t + 1, js],
                                        s0=msc32[:, 2 * t, i:i + 1],
                                        s1=msc32[:, 2 * t + 1, i:i + 1])
                                for lc in range(len(chunks)):
                                    ti, off = divmod(lc * CHUNK, 1024)
                                    per_chunk[lc].append(
                                        (t, th[ti][:, off:off + CHUNK]))
                            # m5: k10,k11. Phase A: DVE pair on cols [0,1024)
                            # + ACT abs on [1024,1536). Phase B: ACT abs.
                            if ph == 0:
                                t5 = abs_t.tile([128, 1024], FP16, tag="at",
                                                name="t5")
                                nc.vector._custom_dve(
                                    ABS_DIFF_PAIR, out=t5[:],
                                    in0=mst[:, 10, 0:1024],
                                    in1=mst[:, 11, 0:1024],
                                    s0=msc32[:, 10, i:i + 1],
                                    s1=msc32[:, 11, i:i + 1])
                                per_chunk[0].append((5, t5[:, 0:CHUNK]))
                                per_chunk[1].append((5, t5[:, CHUNK:1024]))
                                a10 = abs_t.tile([128, 1024], FP16, tag="at",
                                                 name="a10")
                                for kk, cs in ((10, slice(0, CHUNK)),
                                               (11, slice(CHUNK, 1024))):
                                    nc.scalar.activation(
                                        a10[:, cs], mst[:, kk, 1024:1536],
                                        mybir.ActivationFunctionType.Abs,
                                        bias=msc32[:, kk, i:i + 1], scale=-1.0)
                                per_chunk[2].append((5, a10[:, 0:CHUNK]))
                                per_chunk[2].append((5, a10[:, CHUNK:1024]))
                            else:
                                b10 = abs_t.tile([128, 1024], FP16, tag="at",
                                                 name="b10")
                                b11 = abs_t.tile([128, 1024], FP16, tag="at",
                                                 name="b11")
                                for kk, bt in ((10, b10), (11, b11)):
                                    nc.scalar.activation(
                                        bt[:], mst[:, kk, 0:1024],
                                        mybir.ActivationFunctionType.Abs,
                                        bias=msc32[:, kk, i:i + 1], scale=-1.0)
                                for lc in range(len(chunks)):
                                    cs = slice(lc * CHUNK, (lc + 1) * CHUNK)
                                    per_chunk[lc].append((5, b10[:, cs]))
                                    per_chunk[lc].append((5, b11[:, cs]))
                            # m6-9: ACT plains (k12-15), full phase
                            for k in (12, 13, 14, 15):
                                ab = abs_f.tile([128, PH0], FP16, tag="af")
                                nc.scalar.activation(
                                    ab[:, 0:pw], mst[:, k, 0:pw],
                                    mybir.ActivationFunctionType.Abs,
                                    bias=msc32[:, k, i:i + 1], scale=-1.0)
                                for lc in range(len(chunks)):
                                    cs = slice(lc * CHUNK, (lc + 1) * CHUNK)
                                    per_chunk[lc].append((k - 6, ab[:, cs]))
                            mm_ops.append(per_chunk)

                        for jc in chunks:
                            lc = jc - chunks[0]
                            dps = dps_pool.tile([128, CHUNK], F32, tag="dps")
                            for r01 in range(2):
                                orng = slice(64 * r01, 64 * r01 + 64)
                                ops = mm_ops[r01][lc]
                                for oi, (m, rhs_ap) in enumerate(ops):
                                    nc.tensor.matmul(
                                        dps[orng, :], lhsT=masks_sb[:, m, :],
                                        rhs=rhs_ap,
                                        start=(oi == 0),
                                        stop=(oi == len(ops) - 1))
                            e_c = epool.tile([128, CHUNK], FP16, tag="ec")
                            nc.scalar.activation(e_c[:], dps[:],
                                                 mybir.ActivationFunctionType.Exp,
                                                 scale=-1.0,
                                                 accum_out=rs[:, q, jc:jc + 1])
                            if 1 <= jc <= 3:
                                nc.tensor.matmul(
                                    tp_ps[:, (jc - 1) * CHUNK:jc * CHUNK],
                                    lhsT=ident_sb[:], rhs=e_c[:],
                                    start=(q == 0), stop=(q == packs - 1),
                                    skip_group_check=True)

                # ---------------- finish ----------------
                rs_red = l3w.tile([128, packs], F32, tag="rsred")
                nc.vector.tensor_reduce(rs_red[:], rs[:],
                                        axis=mybir.AxisListType.X,
                                        op=mybir.AluOpType.add)
                nc.sync.dma_start(rs_out[:], rs_red[:])
                tpo_sb = l3w.tile([128, TPW], F32, tag="tposb")
                nc.scalar.copy(tpo_sb[:], tp_ps[:])
                nc.sync.dma_start(tpo[:], tpo_sb[:])

    nc.compile()
    return nc


def make_in_maps(x, W1, b1, W2, b2, T, W3, b3, n_cores=N_CORES):
    x = np.asarray(x, np.float32)
    xT_full = np.ascontiguousarray(x.T)                       # [100, 4096]
    W1 = np.asarray(W1, np.float32)
    W2 = np.asarray(W2, np.float32)
    T_perm = np.ascontiguousarray(
        np.asarray(T, np.float32)[:, t_col_perm()])
    w1t = np.ascontiguousarray(W1.T)                           # [100, 200]
    w2t = np.ascontiguousarray(W2.T)                           # [200, 100]
    common = {
        "w1a": np.ascontiguousarray(w1t[:, 0:128]),
        "w1b": np.ascontiguousarray(w1t[:, 128:H1]),
        "b1a": np.ascontiguousarray(np.asarray(b1, np.float32).reshape(H1, 1)[0:128]),
        "b1b": np.ascontiguousarray(np.asarray(b1, np.float32).reshape(H1, 1)[128:H1]),
        "w2a": np.ascontiguousarray(w2t[0:128, :]),
        "w2b": np.ascontiguousarray(w2t[128:H1, :]),
        "b2": np.asarray(b2, np.float32).reshape(H2, 1),
        "tperm": T_perm,
        "masks": make_masks(),
        "ident": np.eye(128, dtype=np.float16),
    }
    in_maps = []
    for c in range(n_cores):
        m = dict(common)
        m["xT"] = np.ascontiguousarray(
            np.roll(xT_full, -c * ROWS, axis=1)[:, 0:JW])
        in_maps.append(m)
    return in_maps


def assemble(core_outs, W3, b3):
    """Combine per-core (rs_out, tpo, ho) into the final [N, 1] output."""
    W3 = np.asarray(W3, np.float64)
    own = np.zeros((N_CORES, ROWS, NB))
    tp = np.zeros((N_CORES, NB, TPW))
    h_full = np.zeros((N, D_IN))
    for c in range(N_CORES):
        rsv = np.asarray(core_outs[c]["rs_out"], np.float64)  # [128, packs]
        tpov = np.asarray(core_outs[c]["tpo"], np.float64)    # [128, TPW]
        hov = np.asarray(core_outs[c]["ho"], np.float64)      # [100, ROWS]
        own[c, 0::2, :] = rsv[0:64, :].T
        own[c, 1::2, :] = rsv[64:128, :].T
        tp[c] = tpov[0:64, :] + tpov[64:128, :]
        h_full[c * ROWS:(c + 1) * ROWS, :] = hov.T
    out_T = np.zeros((N, NB))
    for c in range(N_CORES):
        rows = slice(c * ROWS, (c + 1) * ROWS)
        acc = own[c].copy()
        for d in (1, 2, 3):
            src = (c - d) % N_CORES
            acc += tp[src][:, (d - 1) * ROWS:d * ROWS].T
        out_T[rows] = acc
    z = h_full @ W3[0, :H2] + out_T @ W3[0, H2:] + float(np.asarray(b3).reshape(-1)[0])
    return (1.0 / (1.0 + np.exp(-z))).reshape(N, 1).astype(np.float32)


_NC_CACHE = {}


def kernel(x, W1, b1, W2, b2, T, W3, b3):
    key = "main"
    if key not in _NC_CACHE:
        _NC_CACHE[key] = build_nc()
    nc = _NC_CACHE[key]
    in_maps = make_in_maps(x, W1, b1, W2, b2, T, W3, b3)
    res = run_bass_kernel_spmd(nc, in_maps, list(range(N_CORES)))
    return assemble(res.results, W3, b3)
